# revision 40
# baseline (speedup 1.0000x reference)
"""Distributed Trainium2 Bass kernel for nn_Attention_50139448213963.

Attention layer with per-head QK-layernorm + interleaved RoPE:
  qkv = x @ Wqkv_w.T + Wqkv_b ; q,k = LN_head(q|k) ; q,k = rope(q|k)
  out = softmax(q k^T / sqrt(d)) v ; out = concat_heads @ out_w.T + out_b

Sharding (8 cores): core c -> batch c//4, heads {2*(c%4), 2*(c%4)+1}
(data parallel on B, tensor parallel on heads).  Each core computes QKV
for its 2 heads, attention, and the out-proj partial using its heads'
columns of out_w.  Host sums the 4 partials per batch; out_b and the
v-bias term (exactly foldable through softmax-normalized attention) are
added host-side.  q/k biases would need the on-device rank-1 path
(emit_qk_bias) -- they are zero for this problem.

Engine balance (the point of this revision): the modeled baseline had
PE 145us / ACT 128us busy on a 174us span -- both nearly saturated.
HW A/B: 232us (baseline) -> 186us (this kernel), same-process slope.
  * PE cut: AV runs ONE matmul per k-tile (v without the trailing
    ones-column; M=128 exactly, so the probs stream passes the PE once,
    not twice).  The softmax sums instead accumulate on DVE (two fp16
    adds per pr pair) and reduce at block end with a rank-1 PE matmul +
    DVE reciprocal + GPSIMD partition_broadcast.  (-27us PE, +24us of
    idle DVE.  NOTE: gpsimd partition_all_reduce looked cheaper on the
    cost model but measured ~55us SLOWER on HW; tensor_scalar pow is
    rejected by walrus codegen on both Pool and DVE.)
  * ACT cut: exp batched over k-tile PAIRS (one [P, 2*QC] activation
    from a 2-bank PSUM tile: halves the per-instruction overhead), the
    LN-apply reads PSUM directly (killing the separate eviction copy),
    and 1/sqrt(var+eps) is computed as exp(-0.5*ln(var+eps)) so with the
    Bacc table filter the ONLY act table set is natural_log_exp_and_
    others -- the one load hoists out of the repeat loop instead of
    reloading (2-3x ~1.3us) every iteration.
  * pr/v/sums-acc are fp16 (same PE rate, 8x finer mantissa than bf16).

Per-core dataflow (matmuls 16-bit, fp32 PSUM accumulation):
  1. Input DMA on ONE queue in dependency order (block 0 fine-grained so
     the first QKV matmul starts after ~220KB).
  2. QKV per token-tile-PAIR into a 2-bank PSUM tile; LN stats
     (bn_stats/bn_aggr) read PSUM directly; LN-apply (DVE tensor_scalar
     x*rs + (-mu*rs)) also reads PSUM and writes the bf16 rope input.
     RoPE: a-term cos multiply on DVE, halves-swapped sign-folded
     sin-multiplies on GPSIMD, final add on DVE.  Tile t's apply/rope
     rides behind tile t+1's stats chain (phase2 lag 1).
  3. q,k transposed to [d, tok] via PE transposes, 3 tiles back.
  4. Per (head, 512-wide q chunk): k-tile pairs: two scoresT matmuls
     into the pair PSUM, ONE exp (PSUM->SBUF fp16, scale=1/sqrt(d)),
     the previous pair's two AV matmuls (full 128-row v), and two DVE
     adds into the fp16 sums accumulator.
  5. Block end: rank-1 PE matmul sums the probs accumulator's columns,
     DVE reciprocal of the row, GPSIMD partition_broadcast, one DVE
     multiply normalizes AV from PSUM into bf16.
  6. Out-proj per tok tile accumulating both heads, evict fp16, ONE
     256KB store per tok tile on the sync queue (host upcasts + sums).

Scheduling: engine sequencers are IN-ORDER, so emission order is the
schedule.  Stage 2 software pipelines: block i carries block i-1's
normalize (mid) and the proj chunks of the q-chunk finished one block
ago (work items, all emitted before mid so they bind to the right avn
generation).  PSUM: tag A = 2x 2-bank pair tiles (QKV / scores+exp),
tag B = 2x 1-bank (v pairs / AV accumulator), tag C = 2x 1-bank
(transposes / out-proj) = exactly 8 banks, one pool, no barriers.

`repeat=k` wraps the body in a tc.For_i hardware loop (2 bodies per
iteration) so one NEFF executes the kernel k times back-to-back;
test.py uses the slope between two repeat counts for true device time.
"""

import math
import os
from contextlib import ExitStack

import numpy as np
import ml_dtypes

import concourse.bass as bass
import concourse.tile as tile
from concourse import bacc, mybir
from concourse.bass import ts, ds
from concourse.bass_utils import run_bass_kernel_spmd
from concourse.hw_specs import get_activation_tables
from concourse.masks import make_identity


class Bacc(bacc.Bacc):
    """Bacc whose act-table chooser sees Exp/Ln only in the combined
    natural_log_exp_and_others set.  The stock chooser binds Exp to
    exp_and_others and Ln to natural_log, forcing a ~1.3us table reload on
    EVERY Ln<->Exp alternation (32+ per kernel body).  Hiding the two
    functions from the single-function sets makes every activation resolve
    to one resident set, so after the warm-up load no reload is ever
    emitted.  Set indices are preserved (only membership is filtered), so
    the emitted act_func_set_id still matches act_info.json."""

    def insert_act_table_loads(self):
        has_activation = any(
            isinstance(i, mybir.InstActivation)
            for b in self.main_func.blocks
            for i in b.instructions
        )
        if not has_activation:
            return
        import bass_rust as _bass_rust
        combined = "natural_log_exp_and_others"
        # hide EVERY function this kernel emits from the other sets — the
        # chooser binds each activation to the first set containing its
        # function, so Copy/Identity must also resolve to the combined set
        hide = {mybir.ActivationFunctionType.Exp, mybir.ActivationFunctionType.Ln,
                mybir.ActivationFunctionType.Copy, mybir.ActivationFunctionType.Identity}
        tables = [
            (name, funcs if name == combined else funcs - hide)
            for name, funcs in get_activation_tables(self.m.arch).items()
        ]
        _bass_rust.insert_act_table_loads(self, tables)

F32 = mybir.dt.float32
F16 = mybir.dt.float16
BF16 = mybir.dt.bfloat16

DIM = 1024
HEADS = 8
D = 128  # head dim
B = 2
N = 2048
EPS = 1e-6
HPC = 2  # heads per core
N_CORES = 8
P = 128  # partitions
QC = 512  # q chunk for attention
N_TILES = N // P  # 16
K_IN = DIM // P  # 8 k-tiles over input dim
W_OUT = HPC * 3 * D  # 768 qkv outdims per core
SCALE = 1.0 / math.sqrt(D)
# wqkv block layout (free offsets): q0,k0,q1,k1 then v0,v1
OFF_Q = [0 * D, 2 * D]
OFF_K = [1 * D, 3 * D]
OFF_V = [4 * D, 5 * D]


def build_core_graph(nc, n_tok=N, dtype_mm=BF16, emit_qk_bias=False,
                     shared_rope=True, repeat=1, unroll=2, exp_pairs=True,
                     dual_acc=False):
    """Emit the per-core program. All cores run the same graph (SPMD)."""
    n_tiles = n_tok // P
    assert n_tiles % 2 == 0, "pair-tiled kernel requires an even tile count"
    n_qc = n_tok // QC if n_tok >= QC else 1
    qc = min(QC, n_tok)
    tpq = qc // P  # tok tiles per q chunk
    n_blk = min(4, n_tiles)  # token blocks for the load pipeline
    tpb = n_tiles // n_blk   # tiles per block
    n_tp = n_tiles // 2  # token/k tile pairs
    RW = 2 * D if shared_rope else 4 * D  # rope row width (cos|sin[|cos|sin])

    # ---- dram parameters ----
    xT = nc.dram_tensor("xT", [DIM, n_tok], dtype_mm, kind="ExternalInput").ap()
    wqkv = nc.dram_tensor("wqkv", [DIM, W_OUT], dtype_mm, kind="ExternalInput").ap()
    wout = nc.dram_tensor("wout", [HPC * D, DIM], dtype_mm, kind="ExternalInput").ap()
    rope = nc.dram_tensor("rope", [n_tok, RW], BF16, kind="ExternalInput").ap()
    bqkv = None
    if emit_qk_bias:
        bqkv = nc.dram_tensor("bqkv", [1, W_OUT], F32, kind="ExternalInput").ap()
    out = nc.dram_tensor("out", [n_tok, DIM], F16, kind="ExternalOutput").ap()

    with tile.TileContext(nc) as tc, ExitStack() as ctx:
        const = ctx.enter_context(tc.tile_pool(name="const", bufs=1))
        big = ctx.enter_context(tc.tile_pool(name="big", bufs=1))

        # resident SBUF tensors
        xT_sb = big.tile([P, K_IN, n_tok], dtype_mm, tag="xT_sb")
        wqkv_sb = big.tile([P, K_IN, W_OUT], dtype_mm, tag="wqkv_sb")
        wout_sb = big.tile([P, HPC, DIM], dtype_mm, tag="wout_sb")
        rope_sb = big.tile([P, n_tiles, RW], BF16, tag="rope_sb")
        qT_all = big.tile([P, HPC, n_tok], dtype_mm, tag="qT_all")
        kT_all = big.tile([P, HPC, n_tok], dtype_mm, tag="kT_all")
        v_all = big.tile([P, n_tiles, HPC, D], F16, tag="v_all")
        avn_sb = [big.tile([P, qc], dtype_mm, tag=f"avn{h}", name=f"avn{h}", bufs=min(2, n_qc))
                  for h in range(HPC)]

        ones_row = const.tile([1, P], F32)  # bcast rank-1 lhsT (qk bias only)
        ones_col = const.tile([P, 1], F16)  # lhsT for the probs column sums
        eps_col = const.tile([P, 1], F32)
        ident = const.tile([P, P], dtype_mm)
        bias_sb = const.tile([1, W_OUT], F32) if emit_qk_bias else None

        # single PSUM pool, tags shared across stages (8 banks, no barriers):
        #   A bufs=2 (2-bank pairs): qkv-pair | scores-pair
        #   B bufs=2: v-pair | av     C bufs=2: tp | po
        ps = ctx.enter_context(tc.tile_pool(name="ps", bufs=1, space="PSUM"))
        s1 = ctx.enter_context(tc.tile_pool(name="s1", bufs=6))
        s1small = ctx.enter_context(tc.tile_pool(name="s1small", bufs=8))
        probs = ctx.enter_context(tc.tile_pool(name="probs", bufs=3))
        s2 = ctx.enter_context(tc.tile_pool(name="s2", bufs=2))
        s3 = ctx.enter_context(tc.tile_pool(name="s3", bufs=8))

        def warm_act_tables():
            """Compute eps_col = exp(ln(EPS)) OUTSIDE the repeat loop: the
            act-table-load pass only inserts loads where the set isn't
            resident on every incoming path, so priming the combined set here
            keeps the loop body free of per-iteration table reloads (~1.3us
            each).  Producing a live constant keeps DCE from dropping it."""
            w0 = const.tile([P, 1], F32)
            nc.vector.memset(w0[:], float(math.log(EPS)))
            nc.scalar.activation(eps_col[:], w0[:],
                                 mybir.ActivationFunctionType.Exp)

        def body():
            if emit_qk_bias:
                nc.vector.memset(ones_row[:], 1.0)
            nc.vector.memset(ones_col[:], 1.0)
            make_identity(nc, ident)

            # ---- input loads: ONE queue, few big DMAs, dependency order ----
            # (HWDGE descriptor generation is ~600ns/DMA: batching transfers
            # matters as much as ordering them.)
            xT_r = xT.rearrange("(k p) n -> p k n", p=P)      # [P, K_IN, n_tok]
            rope_r = rope.rearrange("(t p) f -> p t f", p=P)  # [P, n_tiles, RW]
            # block 0 fine-grained so the first QKV matmul starts after ~220KB
            for kk in range(K_IN):
                nc.sync.dma_start(wqkv_sb[:, kk, :], wqkv[ts(kk, P), :])
                nc.sync.dma_start(xT_sb[:, kk, 0:tpb * P], xT[ts(kk, P), 0:tpb * P])
            if emit_qk_bias:
                nc.sync.dma_start(bias_sb[:], bqkv[:])
            nc.sync.dma_start(rope_sb[:, 0:tpb, :], rope_r[:, 0:tpb, :])
            for b in range(1, n_blk):
                tsl = ds(b * tpb * P, tpb * P)
                nc.sync.dma_start(xT_sb[:, :, tsl], xT_r[:, :, tsl])
                nc.sync.dma_start(rope_sb[:, b * tpb:(b + 1) * tpb, :],
                                  rope_r[:, b * tpb:(b + 1) * tpb, :])
            nc.sync.dma_start(wout_sb[:], wout.rearrange("(h p) w -> p h w", p=P))

            # ---------- stage 1: QKV + LN + RoPE ----------
            ro_tiles = {}  # t -> rope-applied bf16 tile awaiting transpose

            def emit_transposes(t):
                ro = ro_tiles.pop(t)  # [P, 2(qk), HPC*D]
                # interleave q/k so consecutive C-slot evictions alternate
                # ACT/DVE: the slot-reuse WAR never waits behind the other
                # eviction queued on the same engine
                for h in range(HPC):
                    for gi, dst in ((0, qT_all), (1, kT_all)):
                        tp = ps.tile([P, P], BF16, tag="C", bufs=2,
                                     name=f"tp{t}{gi}{h}")
                        nc.tensor.transpose(tp[:], ro[:, gi, ts(h, D)], ident[:])
                        if gi == 0:
                            nc.scalar.activation(dst[:, h, ts(t, P)], tp[:],
                                                 mybir.ActivationFunctionType.Copy)
                        else:
                            nc.vector.tensor_copy(dst[:, h, ts(t, P)], tp[:])

            pend = {}  # t -> (ps_pair, half, rs, negmurs) awaiting phase2

            def phase2(t):
                """LN apply + rope for tile t (its stats chain is long done,
                so no engine head-of-line-blocks on these)."""
                ps_pair, u, rs, negmurs = pend.pop(t)
                # LN apply straight from PSUM: x*rs + (-mu*rs) per group via
                # DVE tensor_scalar (per-partition scalar APs), into one
                # [P, 2(qk), 2h*D] bf16 tile so rope runs single wide ops
                qkn = s1.tile([P, 2, HPC * D], BF16, tag="qkn")
                pa = ps_pair[:, u].rearrange("p (s x) -> p s x", x=D)
                for g in range(2):
                    for h in range(HPC):
                        s = 2 * h + g
                        nc.scalar.activation(qkn[:, g, ts(h, D)], pa[:, s, :],
                                             mybir.ActivationFunctionType.Identity,
                                             bias=negmurs[:, s:s + 1],
                                             scale=rs[:, s:s + 1])

                # rope over q and k at once: x4 [P, 2(qk), 2(h), D]; tables
                # broadcast over the head dim (and the qk dim when shared).
                # a = x*cos (DVE), b = halves-swapped x * sign-folded sin
                # (GPSIMD), ro = a + b (DVE).
                x4 = qkn.rearrange("p g (s x) -> p g s x", x=D)
                if shared_rope:
                    ct = rope_sb[:, t, :].rearrange("p (o u x) -> p o u x", o=1, u=1)
                else:
                    ct = rope_sb[:, t, :].rearrange("p (g u x) -> p g u x", g=2, u=1)
                cosT, sinT = ct[:, :, :, 0:D], ct[:, :, :, D:2 * D]
                a = s1.tile([P, 2, HPC * D], BF16, tag="ra")
                a4 = a.rearrange("p g (s x) -> p g s x", x=D)
                bb = s1.tile([P, 2, HPC * D], BF16, tag="rb")
                b4 = bb.rearrange("p g (s x) -> p g s x", x=D)
                i0, i1 = bass.broadcast_tensor_aps(x4[:], cosT)
                nc.vector.tensor_tensor(a4[:], i0, i1, mybir.AluOpType.mult)
                j0, j1 = bass.broadcast_tensor_aps(x4[:, :, :, D // 2:D],
                                                   sinT[:, :, :, 0:D // 2])
                nc.gpsimd.tensor_tensor(b4[:, :, :, 0:D // 2], j0, j1,
                                        mybir.AluOpType.mult)
                j0, j1 = bass.broadcast_tensor_aps(x4[:, :, :, 0:D // 2],
                                                   sinT[:, :, :, D // 2:D])
                nc.gpsimd.tensor_tensor(b4[:, :, :, D // 2:D], j0, j1,
                                        mybir.AluOpType.mult)
                ro = s1.tile([P, 2, HPC * D], dtype_mm, tag="ro")
                nc.vector.tensor_tensor(ro[:], a[:], bb[:], mybir.AluOpType.add)
                ro_tiles[t] = ro

            for pj in range(n_tp):
                # 2-bank pair psum: per tile [q0,k0,q1,k1] (512); v pair (2x256)
                ps_a = ps.tile([P, 2, 4 * D], F32, tag="A", bufs=2, name=f"ps_a{pj}")
                ps_b = ps.tile([P, 2, 2 * D], F32, tag="B", bufs=2, name=f"ps_b{pj}")
                for u in range(2):
                    t = 2 * pj + u
                    # transposes of tile t-3 FIRST: their C-slot evictions
                    # then have this tile's whole QKV burst to drain, instead
                    # of stalling PE mid-tile on the eviction WAR
                    if t >= 3:
                        emit_transposes(t - 3)
                    for kk in range(K_IN):
                        lhsT = xT_sb[:, kk, ts(t, P)]
                        nc.tensor.matmul(ps_a[:, u, :], lhsT, wqkv_sb[:, kk, 0:4 * D],
                                         start=(kk == 0), stop=(kk == K_IN - 1))
                        nc.tensor.matmul(ps_b[:, u, :], lhsT, wqkv_sb[:, kk, 4 * D:W_OUT],
                                         start=(kk == 0), stop=(kk == K_IN - 1))
                    if emit_qk_bias:
                        nc.tensor.matmul(ps_a[:, u, :], ones_row[:, :],
                                         bias_sb[:, 0:4 * D], start=False, stop=True)
                        nc.tensor.matmul(ps_b[:, u, :], ones_row[:, :],
                                         bias_sb[:, 4 * D:W_OUT], start=False, stop=True)

                    # LN stats straight from PSUM (DVE).  The HW verifier
                    # requires exactly one 6-element group per BNStats.
                    stats = s1small.tile([P, 4, 6], F32, tag="stats")
                    mv = s1small.tile([P, 4, 2], F32, tag="mv")
                    pa4 = ps_a[:, u].rearrange("p (s x) -> p s x", x=D)
                    for s in range(4):
                        nc.vector.bn_stats(stats[:, s, :], pa4[:, s, :])
                        nc.vector.bn_aggr(mv[:, s, :], stats[:, s, :])

                    # rs = exp(-0.5*ln(var+eps)): with the Bacc table filter
                    # above this stays inside ONE act table set (walrus
                    # rejects pow on both Pool and DVE, so ACT it is).
                    lnv = s1small.tile([P, 4], F32, tag="lnv")
                    nc.scalar.activation(lnv[:, :], mv[:, :, 1],
                                         mybir.ActivationFunctionType.Ln,
                                         bias=eps_col[:])
                    rs = s1small.tile([P, 4], F32, tag="rs")
                    nc.scalar.activation(rs[:, :], lnv[:, :],
                                         mybir.ActivationFunctionType.Exp,
                                         scale=-0.5)

                    # bias = -(mu*rs) so LN-apply computes (x*rs + bias)
                    negrs = s1small.tile([P, 4], F32, tag="negrs")
                    nc.gpsimd.tensor_scalar(negrs[:, :], rs[:, :], -1.0, None,
                                            mybir.AluOpType.mult)
                    negmurs = s1small.tile([P, 4], F32, tag="negmurs")
                    nc.gpsimd.tensor_tensor(negmurs[:, :], mv[:, :, 0], negrs[:, :],
                                            mybir.AluOpType.mult)
                    pend[t] = (ps_a, u, rs, negmurs)

                    # software pipeline: tile t-1's LN-apply/rope rides
                    # behind tile t's stats chain
                    if t >= 1:
                        phase2(t - 1)

                # v pair evicted in one ACT copy (fp16)
                nc.scalar.activation(
                    v_all[:, 2 * pj:2 * pj + 2, :, :],
                    ps_b.rearrange("p u (h x) -> p u h x", x=D),
                    mybir.ActivationFunctionType.Copy)
            phase2(n_tiles - 1)
            for tt in range(max(0, n_tiles - 3), n_tiles):
                emit_transposes(tt)

            # ---------- stage 2+3: attention + out-projection, sw-pipelined ----------
            av_tiles = {}

            def emit_block(qi, h, mid=None, work=()):
                """scores + exp + AV + sums over all k-tile pairs for (qi, h).
                `mid` fires once mid-loop; `work` items (prev chunk's proj
                pieces) are spread one per pair, all before mid() so they
                bind to the avn generation mid overwrites."""
                av = ps.tile([P, qc], F32, tag="B", bufs=2, name=f"av_{qi}_{h}")
                acc = s2.tile([P, qc], F16, tag="acc", name=f"acc{qi}{h}")
                accB = (s2.tile([P, qc], F16, tag="accB", name=f"accB{qi}{h}")
                        if dual_acc else None)
                mid_j = min(n_tp - 1, 5)
                work = list(work)
                pend_av = None

                def do_av(j, prp):
                    for u in range(2):
                        kt = 2 * j + u
                        nc.tensor.matmul(av[:], v_all[:, kt, h, :], prp[:, u, :],
                                         start=(kt == 0), stop=(kt == n_tiles - 1))

                def do_acc(j, prp):
                    # two fp16 DVE adds per pair into the running sums
                    # (GPSIMD is ~5x slower per element -- keep it off these).
                    # dual_acc keeps TWO independent running sums so the
                    # serial read-modify-write chain is half as deep.
                    if dual_acc:
                        if j == 0:
                            nc.vector.tensor_copy(acc[:], prp[:, 0, :])
                            nc.vector.tensor_copy(accB[:], prp[:, 1, :])
                            return
                        nc.vector.tensor_tensor(acc[:], acc[:], prp[:, 0, :],
                                                mybir.AluOpType.add)
                        nc.vector.tensor_tensor(accB[:], accB[:], prp[:, 1, :],
                                                mybir.AluOpType.add)
                        return
                    if j == 0:
                        nc.vector.tensor_tensor(acc[:], prp[:, 0, :], prp[:, 1, :],
                                                mybir.AluOpType.add)
                        return
                    nc.vector.tensor_tensor(acc[:], acc[:], prp[:, 0, :],
                                            mybir.AluOpType.add)
                    nc.vector.tensor_tensor(acc[:], acc[:], prp[:, 1, :],
                                            mybir.AluOpType.add)

                for j in range(n_tp):
                    if j == mid_j:
                        for w in work:
                            w()
                        work = []
                        if mid is not None:
                            mid()
                    if work:
                        work.pop(0)()
                    scp = ps.tile([P, 2, qc], F32, tag="A", bufs=2,
                                  name=f"sc{qi}{h}{j}")
                    for u in range(2):
                        kt = 2 * j + u
                        nc.tensor.matmul(scp[:, u, :], kT_all[:, h, ts(kt, P)],
                                         qT_all[:, h, ds(qi * qc, qc)],
                                         start=True, stop=True)
                    prp = probs.tile([P, 2, qc], F16, tag="pr")
                    if exp_pairs:
                        nc.scalar.activation(prp[:], scp[:],
                                             mybir.ActivationFunctionType.Exp,
                                             scale=SCALE)
                    else:
                        for u in range(2):
                            nc.scalar.activation(prp[:, u, :], scp[:, u, :],
                                                 mybir.ActivationFunctionType.Exp,
                                                 scale=SCALE)
                    # PE stays busy on the PREVIOUS pair's AV while ACT exps
                    # this pair; DVE sums adds trail one pair behind too.
                    if pend_av is not None:
                        do_av(*pend_av)
                        do_acc(*pend_av)
                    pend_av = (j, prp)
                do_av(*pend_av)
                do_acc(*pend_av)
                if dual_acc:
                    nc.vector.tensor_tensor(acc[:], acc[:], accB[:],
                                            mybir.AluOpType.add)
                av_tiles[(qi, h)] = (av, acc)

            def emit_finish(qi, h):
                """Column sums of the probs accumulator: one rank-1 PE
                matmul (216ns stream), DVE reciprocal of the row, GPSIMD
                partition_broadcast (baseline-proven ops only)."""
                av, acc = av_tiles[(qi, h)]
                sums_ps = ps.tile([1, qc], F32, tag="C", bufs=2,
                                  name=f"sums{qi}{h}")
                nc.tensor.matmul(sums_ps[:], ones_col[:], acc[:],
                                 start=True, stop=True)
                rrow = s2.tile([1, qc], F32, tag="rrow", name=f"rrow{qi}{h}")
                nc.vector.reciprocal(rrow[:], sums_ps[:])
                rcp = s2.tile([P, qc], F32, tag="rcp", name=f"rcp{qi}{h}")
                nc.gpsimd.partition_broadcast(rcp[:], rrow[:])
                av_tiles[(qi, h)] = (av, rcp)

            def emit_normalize(qi, h):
                av, rcp = av_tiles.pop((qi, h))
                nc.vector.tensor_tensor(avn_sb[h][:, :], av[:, :], rcp[:, :],
                                        mybir.AluOpType.mult)

            ot_tiles = {}

            def proj_chunk(qi, ti, c, evict_act=False):
                t = qi * tpq + ti
                po = ps.tile([P, QC], F32, tag="C", bufs=2, name=f"po{t}{c}")
                for h in range(HPC):
                    nc.tensor.matmul(po[:], avn_sb[h][:, ts(ti, P)],
                                     wout_sb[:, h, ts(c, QC)],
                                     start=(h == 0), stop=(h == HPC - 1))
                # both column chunks land in one [P, DIM] tile; a single
                # 256KB store per tok tile keeps the store queue short
                if c == 0:
                    ot_tiles[t] = s3.tile([P, DIM], F16, tag="ot", name=f"ot{t}")
                ot = ot_tiles[t]
                if evict_act:
                    nc.scalar.activation(ot[:, ts(c, QC)], po[:],
                                         mybir.ActivationFunctionType.Copy)
                else:
                    nc.vector.tensor_copy(ot[:, ts(c, QC)], po[:])
                if c == DIM // QC - 1:
                    nc.sync.dma_start(out[ts(t, P), :], ot_tiles.pop(t)[:])

            def proj_work(qi):
                return [lambda ti=ti, c=c: proj_chunk(qi, ti, c)
                        for ti in range(tpq) for c in range(DIM // QC)]

            blocks = [(qi, h) for qi in range(n_qc) for h in range(HPC)]
            for i, (qi, h) in enumerate(blocks):
                mid = (lambda p=blocks[i - 1]: emit_normalize(*p)) if i >= 1 else None
                # proj of chunk qi-1 rides inside block (qi, 1): its avn was
                # completed by the normalize injected into block (qi, 0)
                work = proj_work(qi - 1) if (h == HPC - 1 and qi >= 1) else ()
                emit_block(qi, h, mid=mid, work=work)
                emit_finish(qi, h)

            # tail: only the last block's normalize + last chunk's proj are
            # left.  Interleave them half-chunk-wise and alternate eviction
            # engines (ACT is exp-free here) so the drain pipelines.
            lq, lh = blocks[-1]
            av, rcp = av_tiles.pop((lq, lh))
            for half in range(max(1, tpq // 2)):
                csl = ds(half * 2 * P, min(2 * P, qc))
                nc.vector.tensor_tensor(avn_sb[lh][:, csl], av[:, csl],
                                        rcp[:, csl], mybir.AluOpType.mult)
                for ti in range(half * 2, min(half * 2 + 2, tpq)):
                    for c in range(DIM // QC):
                        proj_chunk(lq, ti, c, evict_act=(c == 1))

        if repeat == 1:
            warm_act_tables()
            body()
        else:
            # unrolled bodies per For_i iteration dilute the loop's
            # all-engine barrier so the measured slope is closer to the
            # true per-execution time
            warm_act_tables()
            u = unroll if repeat % unroll == 0 else 1
            with tc.For_i(0, repeat // u, 1):
                for _ in range(u):
                    body()

    return nc


# ---------------- host side ----------------

def _rope_tables(q_gamma, k_gamma, rope_cos, rope_sin, n_tok=N):
    """Build the on-device rope table(s): [cos|sin] rows, sign of the
    rotate-half folded into sin, gamma folded in, NO head duplication and
    NO score scale (the exp activation applies 1/sqrt(d)).  Returns
    (table [n_tok, 2D or 4D], shared: bool)."""
    bf = ml_dtypes.bfloat16
    perm = np.concatenate([np.arange(0, D, 2), np.arange(1, D, 2)])
    partner = np.concatenate([np.arange(0, D, 2) + 1, np.arange(1, D, 2) - 1])
    sgn = np.concatenate([-np.ones(D // 2, np.float32), np.ones(D // 2, np.float32)])
    cosP = rope_cos[:n_tok, perm]
    sinP = rope_sin[:n_tok, perm]

    def tab(g):
        c = cosP * g[perm][None, :]
        s = (sinP * g[partner][None, :]) * sgn[None, :]
        return np.concatenate([c, s], axis=1)

    shared = bool(np.allclose(q_gamma, k_gamma))
    if shared:
        t = tab(q_gamma)
    else:
        t = np.concatenate([tab(q_gamma), tab(k_gamma)], axis=1)
    return np.ascontiguousarray(t.astype(bf)), shared


def _prep_core_inputs(x, Wqkv_w, Wqkv_b, q_gamma, k_gamma, out_w,
                      rope_cos, rope_sin, n_tok=N):
    """Build the 8 per-core input dicts (numpy, host-side sharding)."""
    bf = ml_dtypes.bfloat16
    # even-first permutation of head_dim (rotate_half becomes a 64-half swap)
    perm = np.concatenate([np.arange(0, D, 2), np.arange(1, D, 2)])
    rope_tab, shared = _rope_tables(q_gamma, k_gamma, rope_cos, rope_sin, n_tok)

    Wr = Wqkv_w.reshape(3, HEADS, D, DIM)
    in_maps = []
    for c in range(N_CORES):
        b = c // 4
        hs = [2 * (c % 4), 2 * (c % 4) + 1]
        xT = np.ascontiguousarray(x[b, :n_tok].T).astype(bf)
        blocks = []
        for h in hs:
            blocks.append(Wr[0, h][perm].T)  # q, dim-permuted  [DIM,128]
            blocks.append(Wr[1, h][perm].T)  # k, dim-permuted
        for h in hs:
            blocks.append(Wr[2, h].T)        # v, natural
        wqkv = np.concatenate(blocks, axis=1).astype(bf)  # [DIM, 768]
        wout = np.concatenate(
            [out_w[:, h * D:(h + 1) * D].T for h in hs], axis=0).astype(bf)  # [256,DIM]
        in_maps.append({
            "xT": xT,
            "wqkv": np.ascontiguousarray(wqkv),
            "wout": np.ascontiguousarray(wout),
            "rope": rope_tab,
        })
    return in_maps, shared


def kernel(x, Wqkv_w, Wqkv_b, q_gamma, q_beta, k_gamma, k_beta,
           out_w, out_b, rope_cos, rope_sin, trace=False, tmpdir=None):
    x = np.asarray(x, np.float32)
    Wqkv_w = np.asarray(Wqkv_w, np.float32)
    Wqkv_b = np.asarray(Wqkv_b, np.float32)
    q_gamma = np.asarray(q_gamma, np.float32)
    q_beta = np.asarray(q_beta, np.float32)
    k_gamma = np.asarray(k_gamma, np.float32)
    k_beta = np.asarray(k_beta, np.float32)
    out_w = np.asarray(out_w, np.float32)
    out_b = np.asarray(out_b, np.float32)
    rope_cos = np.asarray(rope_cos, np.float32)
    rope_sin = np.asarray(rope_sin, np.float32)

    assert np.allclose(q_beta, 0) and np.allclose(k_beta, 0), \
        "nonzero q/k layernorm beta not supported by this kernel build"
    emit_qk_bias = not (np.allclose(Wqkv_b[:DIM], 0) and np.allclose(Wqkv_b[DIM:2 * DIM], 0))

    in_maps, shared = _prep_core_inputs(x, Wqkv_w, Wqkv_b, q_gamma, k_gamma,
                                        out_w, rope_cos, rope_sin)

    nc = Bacc("TRN2", target_bir_lowering=False, debug=False,
              num_devices=N_CORES)
    build_core_graph(nc, n_tok=N, emit_qk_bias=emit_qk_bias, shared_rope=shared)
    nc.compile()

    if emit_qk_bias:
        for c in range(N_CORES):
            hs = [2 * (c % 4), 2 * (c % 4) + 1]
            bq = Wqkv_b[:DIM].reshape(HEADS, D)
            bk = Wqkv_b[DIM:2 * DIM].reshape(HEADS, D)
            perm = np.concatenate([np.arange(0, D, 2), np.arange(1, D, 2)])
            blocks = [np.zeros(0, np.float32)]
            for h in hs:
                blocks += [bq[h][perm], bk[h][perm]]
            blocks += [np.zeros(2 * D, np.float32)]
            in_maps[c]["bqkv"] = np.concatenate(blocks)[None, :].astype(np.float32)

    res = run_bass_kernel_spmd(nc, in_maps, core_ids=list(range(N_CORES)),
                               trace=trace, tmpdir=tmpdir)
    partials = [np.asarray(r["out"], np.float32) for r in res.results]

    # host gather: sum the 4 head-group partials per batch; fold v-bias + out_b
    bv = Wqkv_b[2 * DIM:]
    bias_row = out_b + bv @ out_w.T  # [DIM]
    outp = np.empty((B, N, DIM), np.float32)
    for b in range(B):
        outp[b] = sum(partials[4 * b:4 * b + 4]) + bias_row[None, :]
    kernel.last_exec_time_ns = res.exec_time_ns
    return outp
